# Initial kernel scaffold
#
"""CHRONOSNet (3-layer GAT + classifier) on 8 Trainium2 NeuronCores.

Sharding: nodes partitioned by destination id across 8 cores (graph
parallel). Per layer each core computes its shard of the node table
(attention-logit columns + features), AllGathers the full table (halo
exchange), dma_gathers per-edge rows, and does segment softmax +
aggregation via 128-edge-contraction matmuls against host-built 0/1
segment matrices. BatchNorm stats are AllReduced. Weights replicated.
Layer 2 (concat=False) aggregates input features per head and applies
W2 after aggregation, avoiding the [N, 2048] tensor.
"""

import sys

sys.path.insert(0, "/opt/trn_rl_repo")

import numpy as np

import concourse.bacc as bacc
import concourse.mybir as mybir
import concourse.tile as tile
import bass_rust
from concourse import bass_utils
from concourse.masks import make_identity

NCORES = 8
N, F, HID, NH, C = 20000, 236, 256, 8, 32
FP = 256
ROWW = 320        # table row: [al_src 8 | al_dst 8 | pad 48 | x 256]
ALOFF = 64
TC = 8            # tiles per gather chunk (chunk = one 128-row psum group)
BN_EPS = 1e-5
DEN_EPS = 1e-16

f32 = mybir.dt.float32
f32r = mybir.dt.float32r
i16 = mybir.dt.int16
AF = mybir.ActivationFunctionType
ALU = mybir.AluOpType


def _split_excess_waits(nc, limit=1):
    """This walrus build rejects >limit sem-waits per instruction; move the
    excess onto same-engine NoOps inserted immediately before the carrier."""
    n_new = 0
    for bb in nc.main_func.blocks:
        insts = bb.instructions
        i = 0
        while i < len(insts):
            ins = insts[i]
            si = ins.sync_info
            if si is not None and si.on_wait and len(si.on_wait) > limit:
                waits = list(si.on_wait)
                extra, keep = waits[:-limit], waits[-limit:]
                new_insts = []
                for j in range(0, len(extra), limit):
                    nsi = bass_rust.SyncInfo(on_wait=extra[j:j + limit],
                                             on_update=[])
                    new_insts.append(mybir.InstNoOp(
                        name=f"I-ws-{n_new}-{i}-{id(ins) % 9973}",
                        engine=ins.engine, ins=[], outs=[], sync_info=nsi))
                    n_new += 1
                ins.sync_info = bass_rust.SyncInfo(
                    on_wait=keep, on_update=list(si.on_update))
                for k, nd in enumerate(new_insts):
                    insts.insert(i + k, nd)
                i += len(new_insts)
            i += 1
    return n_new


# ------------------------------------------------------------- host packing
def _pack_graph(edge_index):
    e0 = np.asarray(edge_index[0], dtype=np.int64)
    e1 = np.asarray(edge_index[1], dtype=np.int64)
    loops = np.arange(N, dtype=np.int64)
    src = np.concatenate([e0, loops])
    dst = np.concatenate([e1, loops])
    order = np.argsort(dst, kind="stable")
    src_s = src[order]
    deg = np.bincount(dst, minlength=N)
    assert deg.max() <= 128, "node degree exceeds one tile"
    starts = np.zeros(N + 1, np.int64)
    np.cumsum(deg, out=starts[1:])

    NPC = N // NCORES
    assign = []
    tmax = 0
    for c in range(NCORES):
        tiles, cur, cur_e = [], [], 0
        for d in range(c * NPC, (c + 1) * NPC):
            g = int(deg[d])
            if cur and (cur_e + g > 128 or len(cur) == 16):
                tiles.append(cur)
                cur, cur_e = [], 0
            cur.append(d)
            cur_e += g
        if cur:
            tiles.append(cur)
        assign.append(tiles)
        tmax = max(tmax, len(tiles))
    # T: multiple of 2*TC so chunks and layer-2 superblocks divide evenly
    T = ((tmax + 2 * TC - 1) // (2 * TC)) * (2 * TC)
    NPAD = T * 16
    NCH = T // TC
    assert NCORES * NPAD < 32768, "padded node ids must fit int16"

    pid = np.full(N, -1, np.int64)
    for c in range(NCORES):
        for t, nodes in enumerate(assign[c]):
            for s, d in enumerate(nodes):
                pid[d] = c * NPAD + t * 16 + s
    assert (pid >= 0).all()

    idx1 = np.zeros((NCORES, T, 128), np.int16)
    idx2 = np.zeros((NCORES, T, 128), np.int16)
    oh = np.zeros((NCORES, 128, T, 16), np.float32)
    for c in range(NCORES):
        for t, nodes in enumerate(assign[c]):
            p = 0
            for s, d in enumerate(nodes):
                for e in range(starts[d], starts[d + 1]):
                    idx1[c, t, p] = pid[src_s[e]]
                    idx2[c, t, p] = pid[d]
                    oh[c, p, t, s] = 1.0
                    p += 1
            assert p <= 128

    def wrap(idx):
        w = np.zeros((NCORES, 128, NCH, TC * 8), np.int16)
        for c in range(NCORES):
            flat = idx[c].reshape(NCH, TC * 128)
            for ch in range(NCH):
                w[c, :, ch, :] = np.tile(flat[ch].reshape(-1, 16).T, (8, 1))
        return w

    return dict(T=T, NPAD=NPAD, NCH=NCH, pid=pid,
                idx1=wrap(idx1), idx2=wrap(idx2), oh=oh)


def _prep_weights(inp):
    w = {}
    Wp = np.zeros((FP, HID), np.float32)
    Wp[:F] = np.asarray(inp["Wp"])
    w["Wp"] = Wp
    w["bp"] = np.asarray(inp["bp"]).reshape(1, HID)

    for l, CC in ((0, C), (1, C), (2, HID)):
        W = np.asarray(inp[f"W{l}"])
        a_s = np.asarray(inp[f"as{l}"])
        a_d = np.asarray(inp[f"ad{l}"])
        As = np.einsum("khc,hc->kh", W.reshape(HID, NH, CC), a_s)
        Ad = np.einsum("khc,hc->kh", W.reshape(HID, NH, CC), a_d)
        wall = np.zeros((HID, ROWW), np.float32)
        wall[:, 0:8] = As
        wall[:, 8:16] = Ad
        wall[:, ALOFF:] = W if l < 2 else np.eye(HID, dtype=np.float32)
        w[f"Wall{l}"] = wall
        w[f"g{l}"] = np.asarray(inp[f"g{l}"]).reshape(1, HID)
        w[f"be{l}"] = np.asarray(inp[f"be{l}"]).reshape(1, HID)

    W2 = np.asarray(inp["W2"]).reshape(HID, NH, HID)
    W2s = np.zeros((128, 16, 2, 128), np.float32)
    for kt in range(16):
        h, kc = kt // 2, kt % 2
        for ch in range(2):
            W2s[:, kt, ch, :] = W2[kc * 128:(kc + 1) * 128, h,
                                   ch * 128:(ch + 1) * 128] / NH
    w["W2s"] = W2s
    # feature-major per-partition forms for layer-2 BN
    g2 = np.asarray(inp["g2"]).reshape(2, 128).T.copy()
    be2 = np.asarray(inp["be2"]).reshape(2, 128).T.copy()
    w["g2fm"], w["be2fm"] = g2, be2

    cW1 = np.asarray(inp["cW1"])
    w["cW1"] = cW1.reshape(4, 128, 2, 128).transpose(1, 0, 2, 3).copy()
    cW2 = np.asarray(inp["cW2"])
    w["cW2"] = cW2.reshape(2, 128, 128).transpose(1, 0, 2).copy()
    w["cW3"] = np.asarray(inp["cW3"])
    w["cb1"] = np.asarray(inp["cb1"]).reshape(2, 128).T.copy()   # [128, 2]
    w["cb2"] = np.asarray(inp["cb2"]).reshape(128, 1).copy()     # [128, 1]
    w["cb3"] = np.asarray(inp["cb3"]).reshape(2, 1).copy()       # [2, 1]
    return w


# ------------------------------------------------------------ device program
def _build_program(T, NPAD, NCH, debug=False):
    NG = NPAD // 128     # 128-row node groups (node-major)
    NB = NPAD // 512     # 512-col node blocks (feature-major)
    NSB = T // (2 * TC)  # layer-2 projection superblocks (16 tiles each)
    nc = bacc.Bacc("TRN2")

    d_x = nc.dram_tensor("x_fm", [FP, NPAD], f32r, kind="ExternalInput")
    d_idx1 = nc.dram_tensor("idx1", [128, NCH, TC * 8], i16, kind="ExternalInput")
    d_idx2 = nc.dram_tensor("idx2", [128, NCH, TC * 8], i16, kind="ExternalInput")
    d_oh = nc.dram_tensor("oh", [128, T, 16], f32r, kind="ExternalInput")
    d_Wp = nc.dram_tensor("Wp", [FP, HID], f32r, kind="ExternalInput")
    d_bp = nc.dram_tensor("bp", [1, HID], f32, kind="ExternalInput")
    d_wall = [nc.dram_tensor(f"Wall{l}", [HID, ROWW], f32r, kind="ExternalInput")
              for l in range(3)]
    d_g = [nc.dram_tensor(f"g{l}", [1, HID], f32, kind="ExternalInput")
           for l in range(2)]
    d_be = [nc.dram_tensor(f"be{l}", [1, HID], f32, kind="ExternalInput")
            for l in range(2)]
    d_g2 = nc.dram_tensor("g2fm", [128, 2], f32, kind="ExternalInput")
    d_be2 = nc.dram_tensor("be2fm", [128, 2], f32, kind="ExternalInput")
    d_W2s = nc.dram_tensor("W2s", [128, 16, 2, 128], f32r, kind="ExternalInput")
    d_cW1 = nc.dram_tensor("cW1", [128, 4, 2, 128], f32r, kind="ExternalInput")
    d_cW2 = nc.dram_tensor("cW2", [128, 2, 128], f32r, kind="ExternalInput")
    d_cW3 = nc.dram_tensor("cW3", [128, 2], f32r, kind="ExternalInput")
    d_cb1 = nc.dram_tensor("cb1", [128, 2], f32, kind="ExternalInput")
    d_cb2 = nc.dram_tensor("cb2", [128, 1], f32, kind="ExternalInput")
    d_cb3 = nc.dram_tensor("cb3", [2, 1], f32, kind="ExternalInput")
    d_out = nc.dram_tensor("out", [2, NPAD], f32, kind="ExternalOutput")
    if debug:
        d_dbg_h0 = nc.dram_tensor("dbg_h0", [128, NPAD // 128, HID], f32,
                                  kind="ExternalOutput")
        d_dbg_u0 = nc.dram_tensor("dbg_u0", [128, NPAD // 128, 264], f32,
                                  kind="ExternalOutput")
        d_dbg_h1 = nc.dram_tensor("dbg_h1", [128, NPAD // 128, HID], f32,
                                  kind="ExternalOutput")
        d_dbg_h2 = nc.dram_tensor("dbg_h2", [128, NPAD // 128, HID], f32,
                                  kind="ExternalOutput")
        d_dbg_u2 = nc.dram_tensor("dbg_u2", [128, 2, NPAD], f32,
                                  kind="ExternalOutput")
        d_dbg_hf = nc.dram_tensor("dbg_hf", [128, 2, NPAD], f32,
                                  kind="ExternalOutput")

    RG = [list(range(NCORES))]

    with tile.TileContext(nc) as tc, \
         tc.tile_pool(name="const", bufs=1) as const, \
         tc.tile_pool(name="sb", bufs=2) as sbuf, \
         tc.tile_pool(name="sb1", bufs=1) as sb1, \
         tc.tile_pool(name="big", bufs=1) as big, \
         tc.tile_pool(name="psA", bufs=2, space="PSUM") as psA, \
         tc.tile_pool(name="psB", bufs=2, space="PSUM") as psB, \
         tc.tile_pool(name="dram", bufs=1, space="DRAM") as dram:

        # ---------------- constants
        ident_t = const.tile([128, 128], f32, tag="ident")
        make_identity(nc, ident_t[:])
        ident = ident_t[:]
        ones_t = const.tile([128, 1], f32, tag="ones")
        nc.gpsimd.memset(ones_t[:], 1.0)
        ones_col = ones_t[:].bitcast(f32r)
        idx1_sb = const.tile([128, NCH, TC * 8], i16, tag="idx1")
        nc.sync.dma_start(idx1_sb[:], d_idx1[:])
        idx2_sb = const.tile([128, NCH, TC * 8], i16, tag="idx2")
        nc.sync.dma_start(idx2_sb[:], d_idx2[:])
        Wp_sb = const.tile([128, 2, HID], f32r, tag="Wp")
        nc.sync.dma_start(Wp_sb[:],
                          d_Wp.ap().rearrange("(kc k) w -> k kc w", k=128))
        W2s_sb = const.tile([128, 16, 2, 128], f32r, tag="W2s")
        nc.sync.dma_start(W2s_sb[:], d_W2s[:])
        cW1_sb = const.tile([128, 4, 2, 128], f32r, tag="cW1")
        nc.sync.dma_start(cW1_sb[:], d_cW1[:])
        cW2_sb = const.tile([128, 2, 128], f32r, tag="cW2")
        nc.sync.dma_start(cW2_sb[:], d_cW2[:])
        cW3_sb = const.tile([128, 2], f32r, tag="cW3")
        nc.sync.dma_start(cW3_sb[:], d_cW3[:])
        gam_sb = const.tile([1, 2, HID], f32, tag="gam")
        bet_sb = const.tile([1, 2, HID], f32, tag="bet")
        for l in range(2):
            nc.sync.dma_start(gam_sb[:, l, :], d_g[l][:])
            nc.sync.dma_start(bet_sb[:, l, :], d_be[l][:])
        g2_sb = const.tile([128, 2], f32, tag="g2f")
        nc.sync.dma_start(g2_sb[:], d_g2[:])
        be2_sb = const.tile([128, 2], f32, tag="be2f")
        nc.sync.dma_start(be2_sb[:], d_be2[:])
        cb1_sb = const.tile([128, 2], f32, tag="cb1")
        nc.sync.dma_start(cb1_sb[:], d_cb1[:])
        cb2_sb = const.tile([128, 1], f32, tag="cb2")
        nc.sync.dma_start(cb2_sb[:], d_cb2[:])
        cb3_sb = const.tile([2, 1], f32, tag="cb3")
        nc.sync.dma_start(cb3_sb[:], d_cb3[:])
        bp1 = const.tile([1, HID], f32, tag="bp1")
        nc.sync.dma_start(bp1[:], d_bp[:])
        bp_rep = const.tile([128, HID], f32, tag="bpr")
        nc.gpsimd.partition_broadcast(bp_rep[:], bp1[:])

        # ---------------- persistent state
        h_nm = big.tile([128, NG, HID], f32r, tag="h_nm")   # current h
        ht_dram = dram.tile([HID, NPAD], f32r, tag="ht")    # h_temporal (fm)

        # ---------------- input projection: h = x @ Wp + bp (node-major)
        for g in range(NG):
            xst = sbuf.tile([128, 2, 128], f32r, tag="xst")
            nc.sync.dma_start(xst[:], d_x.ap().rearrange(
                "(kc k) n -> k kc n", k=128)[:, :, g * 128:(g + 1) * 128])
            pm = psA.tile([128, 512], f32, tag="pA")
            for kc in range(2):
                nc.tensor.matmul(pm[:, 0:HID], xst[:, kc, :], Wp_sb[:, kc, :],
                                 start=(kc == 0), stop=(kc == 1))
            nc.vector.tensor_add(h_nm[:, g, :], pm[:, 0:HID], bp_rep[:])
            # h_temporal, feature-major, to DRAM
            for kc in range(2):
                pt = psB.tile([128, 128], f32, tag="pB")
                nc.tensor.transpose(pt[:], h_nm[:, g, kc * 128:(kc + 1) * 128].bitcast(f32),
                                    ident)
                hst = sbuf.tile([128, 128], f32r, tag="hst")
                nc.vector.tensor_copy(hst[:], pt[:])
                nc.sync.dma_start(
                    ht_dram[kc * 128:(kc + 1) * 128, g * 128:(g + 1) * 128],
                    hst[:])

        if debug:
            nc.sync.dma_start(d_dbg_h0[:], h_nm[:].bitcast(f32))

        def stat_allreduce(stats, tag):
            if not isinstance(stats, list):
                stats = [stats]
            n = len(stats)
            w = stats[0].shape[-1]
            p = stats[0].shape[0]
            sin = dram.tile([n * p, w], f32, tag=f"ari{tag}")
            sout = dram.tile([n * p, w], f32, tag=f"aro{tag}")
            for k, s in enumerate(stats):
                nc.sync.dma_start(sin[k * p:(k + 1) * p, :], s[:])
            nc.gpsimd.collective_compute(
                "AllReduce", ALU.add, ins=[sin.opt()], outs=[sout.opt()],
                replica_groups=RG)
            for k, s in enumerate(stats):
                nc.sync.dma_start(s[:], sout[k * p:(k + 1) * p, :])

        # ================= GAT layers =================
        for l in range(3):
            # ---- shard of the node table: xrow = h.T-block @ Wall_l
            wall_sb = sbuf.tile([128, 2, ROWW], f32r, tag="wall")
            nc.sync.dma_start(
                wall_sb[:],
                d_wall[l].ap().rearrange("(kc k) w -> k kc w", k=128))
            xloc = dram.tile([NPAD, ROWW], f32r, tag=f"xloc{l}")
            for g in range(NG):
                hTst = sbuf.tile([128, 2, 128], f32r, tag="hTst")
                for kc in range(2):
                    pt = psB.tile([128, 128], f32, tag="pB")
                    nc.tensor.transpose(
                        pt[:], h_nm[:, g, kc * 128:(kc + 1) * 128].bitcast(f32),
                        ident)
                    nc.vector.tensor_copy(hTst[:, kc, :], pt[:])
                pm = psA.tile([128, 512], f32, tag="pA")
                for kc in range(2):
                    nc.tensor.matmul(pm[:, 0:ROWW], hTst[:, kc, :],
                                     wall_sb[:, kc, :],
                                     start=(kc == 0), stop=(kc == 1))
                xr = sbuf.tile([128, ROWW], f32r, tag="xrow")
                nc.vector.tensor_copy(xr[:], pm[:, 0:ROWW])
                nc.sync.dma_start(xloc[g * 128:(g + 1) * 128, :], xr[:])
            xtab = dram.tile([NCORES * NPAD, ROWW], f32r, tag=f"xtab{l}")
            nc.gpsimd.collective_compute(
                "AllGather", ALU.bypass, ins=[xloc.opt()], outs=[xtab.opt()],
                replica_groups=RG)

            # ---- gather chunks + segment softmax + aggregation
            if l < 2:
                u_sb = big.tile([128, NG, 264], f32, tag="u_sb")
            else:
                u2_fm = big.tile([128, 2, NPAD], f32, tag="u_sb")
            g2l = None
            for ch in range(NCH):
                ohc = sbuf.tile([128, TC, 16], f32r, tag="ohc")
                nc.sync.dma_start(ohc[:], d_oh[:, ch * TC:(ch + 1) * TC, :])
                gt = sbuf.tile([128, TC, ROWW], f32r, tag="gmain")
                nc.gpsimd.dma_gather(gt[:], xtab[:], idx1_sb[:, ch, :],
                                     TC * 128, TC * 128, ROWW)
                g2t = sbuf.tile([128, TC, 64], f32, tag="gal")
                nc.gpsimd.dma_gather(g2t[:], xtab.opt()[:, 0:64].bitcast(f32),
                                     idx2_sb[:, ch, :], TC * 128, TC * 128, 64,
                                     elem_step=ROWW)
                lg = sbuf.tile([128, TC, 8], f32, tag="lg")
                nc.vector.tensor_add(lg[:], gt[:, :, 0:8].bitcast(f32),
                                     g2t[:, :, 8:16])
                nc.vector.scalar_tensor_tensor(lg[:], lg[:], 0.2, lg[:],
                                               ALU.mult, ALU.max)
                ex = sbuf.tile([128, TC, 8], f32, tag="ex")
                nc.scalar.activation(ex[:], lg[:], AF.Exp)
                if l < 2:
                    y = sbuf.tile([128, TC, 264], f32r, tag="y")
                    nc.vector.tensor_mul(
                        y[:, :, 0:256].rearrange("p t (h c) -> p t h c", h=8),
                        gt[:, :, ALOFF:].bitcast(f32)
                          .rearrange("p t (h c) -> p t h c", h=8),
                        ex[:].unsqueeze(3).broadcast_to([128, TC, 8, 32]))
                    nc.vector.tensor_copy(y[:, :, 256:264], ex[:])
                    ohg = sb1.tile([128, TC, 128], f32r, tag="ohg")
                    nc.vector.memset(ohg[:].bitcast(f32), 0.0)
                    for j in range(TC):
                        nc.vector.tensor_copy(
                            ohg[:, j, j * 16:(j + 1) * 16], ohc[:, j, :])
                    pu = psA.tile([128, 512], f32, tag="pA")
                    for j in range(TC):
                        nc.tensor.matmul(pu[:, 0:264], ohg[:, j, :],
                                         y[:, j, :], start=(j == 0),
                                         stop=(j == TC - 1))
                    nc.vector.tensor_copy(u_sb[:, ch, :].bitcast(f32r), pu[:, 0:264])
                else:
                    ohex = sbuf.tile([128, TC, 16, 8], f32r, tag="y")
                    nc.vector.tensor_mul(
                        ohex[:],
                        ohc[:].unsqueeze(3).broadcast_to([128, TC, 16, 8]),
                        ex[:].unsqueeze(2).broadcast_to([128, TC, 16, 8]))
                    if ch % 2 == 0:
                        g2l = big.tile([128, 2 * TC, 2, 16, 8], f32r,
                                       tag="g2l")
                    for j in range(TC):
                        tt = (ch % 2) * TC + j
                        pg = psA.tile([128, 512], f32, tag="pA")
                        lhs = ohex[:, j, :, :].rearrange("p s h -> p (s h)")
                        nc.tensor.matmul(pg[:, 0:256], lhs, gt[:, j, ALOFF:],
                                         start=True, stop=True)
                        nc.tensor.matmul(pg[:, 256:257],
                                         lhs.bitcast(f32), ones_t[:],
                                         start=True, stop=True)
                        rden = sbuf.tile([128, 1], f32, tag="rden")
                        nc.vector.tensor_scalar_add(rden[:], pg[:, 256:257],
                                                    DEN_EPS)
                        nc.vector.reciprocal(rden[:], rden[:])
                        gn = sbuf.tile([128, 256], f32r, tag="gn")
                        nc.vector.tensor_scalar(gn[:], pg[:, 0:256], rden[:],
                                                None, ALU.mult)
                        for kc in range(2):
                            pt = psB.tile([128, 128], f32, tag="pB")
                            nc.tensor.transpose(
                                pt[:], gn[:, kc * 128:(kc + 1) * 128]
                                    .bitcast(f32), ident)
                            nc.vector.tensor_copy(
                                g2l[:, tt, kc, :, :]
                                    .rearrange("p s h -> p (s h)"), pt[:])
                    if ch % 2 == 1:
                        sb_i = ch // 2
                        for chh in range(2):
                            pm = psA.tile([128, 512], f32, tag="pA")
                            for kt in range(16):
                                rhs = g2l[:, :, kt % 2, :, kt // 2]
                                nc.tensor.matmul(
                                    pm[:, 0:256], W2s_sb[:, kt, chh, :], rhs,
                                    start=(kt == 0), stop=(kt == 15))
                            nc.vector.tensor_copy(
                                u2_fm[:, chh, sb_i * 256:(sb_i + 1) * 256]
                                    .bitcast(f32r),
                                pm[:, 0:256])

            if debug and l == 0:
                nc.sync.dma_start(d_dbg_u0[:], u_sb[:].bitcast(f32))
            if debug and l == 2:
                nc.sync.dma_start(d_dbg_u2[:], u2_fm[:])
            # ---- postprocess
            if l < 2:
                rden = sbuf.tile([128, NG, 8], f32, tag="rdnA")
                nc.vector.tensor_scalar_add(rden[:], u_sb[:, :, 256:264],
                                            DEN_EPS)
                nc.vector.reciprocal(rden[:], rden[:])
                ubv = u_sb[:, :, 0:256].bitcast(f32r)
                nc.vector.tensor_mul(
                    ubv.rearrange("p g (h c) -> p g h c", h=8),
                    ubv.rearrange("p g (h c) -> p g h c", h=8),
                    rden[:].unsqueeze(3).broadcast_to([128, NG, 8, 32]))
                # stats: sum and sumsq over all node slots (dummies are 0)
                pst = psA.tile([128, 512], f32, tag="pA")
                pst2 = psA.tile([128, 512], f32, tag="pA")
                for g in range(NG):
                    nc.tensor.matmul(pst[0:1, 0:HID], ones_col,
                                     ubv[:, g, :], start=(g == 0),
                                     stop=(g == NG - 1))
                for b in range(NG // 4):
                    sq = sb1.tile([128, 4, 256], f32r, tag="scr1")
                    nc.vector.tensor_mul(sq[:], ubv[:, b * 4:(b + 1) * 4, :],
                                         ubv[:, b * 4:(b + 1) * 4, :])
                    for gg in range(4):
                        g = b * 4 + gg
                        nc.tensor.matmul(pst2[0:1, 0:HID], ones_col,
                                         sq[:, gg, :], start=(g == 0),
                                         stop=(g == NG - 1))
                stat_s = sbuf.tile([1, HID], f32, tag="stat_s")
                stat_q = sbuf.tile([1, HID], f32, tag="stat_q")
                nc.vector.tensor_copy(stat_s[:], pst[0:1, 0:HID])
                nc.vector.tensor_copy(stat_q[:], pst2[0:1, 0:HID])
                stat_allreduce([stat_s, stat_q], f"l{l}")
                mu = sbuf.tile([1, HID], f32, tag="mu")
                nc.vector.tensor_scalar_mul(mu[:], stat_s[:], 1.0 / N)
                var = sbuf.tile([1, HID], f32, tag="var")
                nc.vector.tensor_scalar_mul(var[:], stat_q[:], 1.0 / N)
                musq = sbuf.tile([1, HID], f32, tag="musq")
                nc.vector.tensor_mul(musq[:], mu[:], mu[:])
                nc.vector.tensor_tensor(var[:], var[:], musq[:],
                                        op=ALU.subtract)
                rstd = sbuf.tile([1, HID], f32, tag="rstd")
                nc.vector.tensor_scalar_add(var[:], var[:], BN_EPS)
                nc.scalar.activation(rstd[:], var[:], AF.Sqrt)
                nc.vector.reciprocal(rstd[:], rstd[:])
                A1 = sbuf.tile([1, HID], f32, tag="A1")
                nc.vector.tensor_mul(A1[:], rstd[:], gam_sb[:, l, :])
                B1 = sbuf.tile([1, HID], f32, tag="B1")
                nc.vector.tensor_mul(B1[:], mu[:], A1[:])
                nc.vector.tensor_tensor(B1[:], bet_sb[:, l, :], B1[:],
                                        op=ALU.subtract)
                Ar = sb1.tile([128, HID], f32, tag="Ar")
                nc.gpsimd.partition_broadcast(Ar[:], A1[:])
                Br = sb1.tile([128, HID], f32, tag="Br")
                nc.gpsimd.partition_broadcast(Br[:], B1[:])
                for b in range(NG // 4):
                    sl = slice(b * 4, (b + 1) * 4)
                    bn = sb1.tile([128, 4, 256], f32, tag="scr1")
                    nc.vector.tensor_mul(
                        bn[:], ubv[:, sl, :],
                        Ar[:].unsqueeze(1).broadcast_to([128, 4, HID]))
                    nc.vector.tensor_add(
                        bn[:], bn[:],
                        Br[:].unsqueeze(1).broadcast_to([128, 4, HID]))
                    # elu(x) = relu(x) + exp(min(x,0)) - 1
                    r_ = sb1.tile([128, 4, 256], f32, tag="scr2")
                    nc.scalar.activation(r_[:], bn[:], AF.Relu)
                    nc.vector.tensor_scalar_min(bn[:], bn[:], 0.0)
                    nc.scalar.activation(bn[:], bn[:], AF.Exp)
                    nc.vector.tensor_add(bn[:], bn[:], r_[:])
                    # h_new = (bn - 1) + h_prev, in place on h_nm
                    nc.vector.scalar_tensor_tensor(
                        h_nm[:, sl, :], bn[:], -1.0, h_nm[:, sl, :],
                        ALU.add, ALU.add)
                if debug:
                    nc.sync.dma_start((d_dbg_h1 if l == 0 else d_dbg_h2)[:],
                                      h_nm[:].bitcast(f32))
            else:
                # ---- BN2 (feature-major; dummy cols are exactly 0)
                st2 = sbuf.tile([128, 4], f32, tag="st2")
                nc.vector.reduce_sum(st2[:, 0:2].unsqueeze(2), u2_fm[:],
                                     axis=mybir.AxisListType.X)
                sq2 = big.tile([128, 2, NPAD], f32, tag="g2l")
                nc.vector.tensor_mul(sq2[:], u2_fm[:], u2_fm[:])
                nc.vector.reduce_sum(st2[:, 2:4].unsqueeze(2), sq2[:],
                                     axis=mybir.AxisListType.X)
                stat_allreduce(st2, "l2")
                mu2 = sbuf.tile([128, 2], f32, tag="mu2")
                nc.vector.tensor_scalar_mul(mu2[:], st2[:, 0:2], 1.0 / N)
                var2 = sbuf.tile([128, 2], f32, tag="var2")
                nc.vector.tensor_scalar_mul(var2[:], st2[:, 2:4], 1.0 / N)
                m2sq = sbuf.tile([128, 2], f32, tag="m2sq")
                nc.vector.tensor_mul(m2sq[:], mu2[:], mu2[:])
                nc.vector.tensor_tensor(var2[:], var2[:], m2sq[:],
                                        op=ALU.subtract)
                rstd2 = sbuf.tile([128, 2], f32, tag="rstd2")
                nc.vector.tensor_scalar_add(var2[:], var2[:], BN_EPS)
                nc.scalar.activation(rstd2[:], var2[:], AF.Sqrt)
                nc.vector.reciprocal(rstd2[:], rstd2[:])
                A2 = sbuf.tile([128, 2], f32, tag="A2")
                nc.vector.tensor_mul(A2[:], rstd2[:], g2_sb[:])
                B2 = sbuf.tile([128, 2], f32, tag="B2")
                nc.vector.tensor_mul(B2[:], mu2[:], A2[:])
                nc.vector.tensor_tensor(B2[:], be2_sb[:], B2[:],
                                        op=ALU.subtract)
                hfin = u2_fm[:].bitcast(f32r)
                for chh in range(2):
                    nc.vector.tensor_scalar(
                        hfin[:, chh, :], u2_fm[:, chh, :],
                        A2[:, chh:chh + 1], B2[:, chh:chh + 1],
                        ALU.mult, ALU.add)

        if debug:
            nc.sync.dma_start(d_dbg_hf[:], u2_fm[:])
        # ================= classifier (feature-major) =================
        z1 = big.tile([128, 2, NPAD], f32r, tag="g2l")
        for b in range(NB):
            htst = sb1.tile([128, 2, 512], f32r, tag="htst")
            nc.sync.dma_start(htst[:], ht_dram.opt().rearrange(
                "(kc k) n -> k kc n", k=128)[:, :, b * 512:(b + 1) * 512])
            for mh in range(2):
                pm = psA.tile([128, 512], f32, tag="pA")
                for kc in range(4):
                    rhs = (hfin[:, kc, b * 512:(b + 1) * 512] if kc < 2
                           else htst[:, kc - 2, :])
                    nc.tensor.matmul(pm[:], cW1_sb[:, kc, mh, :], rhs,
                                     start=(kc == 0), stop=(kc == 3))
                nc.vector.tensor_scalar(
                    z1[:, mh, b * 512:(b + 1) * 512], pm[:],
                    cb1_sb[:, mh:mh + 1], 0.0, ALU.add, ALU.max)
        z2 = big.tile([128, NPAD], f32r, tag="h_nm")
        for b in range(NB):
            pm = psA.tile([128, 512], f32, tag="pA")
            for kc in range(2):
                nc.tensor.matmul(pm[:], cW2_sb[:, kc, :],
                                 z1[:, kc, b * 512:(b + 1) * 512],
                                 start=(kc == 0), stop=(kc == 1))
            nc.vector.tensor_scalar(z2[:, b * 512:(b + 1) * 512], pm[:],
                                    cb2_sb[:], 0.0, ALU.add, ALU.max)
        for b in range(NB):
            pm = psA.tile([128, 512], f32, tag="pA")
            nc.tensor.matmul(pm[0:2, :], cW3_sb[:],
                             z2[:, b * 512:(b + 1) * 512],
                             start=True, stop=True)
            z3 = sbuf.tile([2, 512], f32, tag="z3")
            nc.vector.tensor_scalar_add(z3[:], pm[0:2, :], cb3_sb[:])
            nc.sync.dma_start(d_out.ap()[:, b * 512:(b + 1) * 512], z3[:])

    nc.compile()
    _split_excess_waits(nc)
    return nc


_CACHE = {}


def kernel(**inputs):
    meta = _pack_graph(inputs["edge_index"])
    T, NPAD, NCH = meta["T"], meta["NPAD"], meta["NCH"]
    key = (T, NPAD, NCH)
    if key not in _CACHE:
        _CACHE[key] = _build_program(T, NPAD, NCH)
    nc = _CACHE[key]

    w = _prep_weights(inputs)
    x = np.asarray(inputs["x"], dtype=np.float32)
    pid = meta["pid"]
    NPC = N // NCORES

    in_maps = []
    for c in range(NCORES):
        x_fm = np.zeros((FP, NPAD), np.float32)
        lp = pid[c * NPC:(c + 1) * NPC] - c * NPAD
        x_fm[:F, lp] = x[c * NPC:(c + 1) * NPC].T
        m = {"x_fm": x_fm, "idx1": meta["idx1"][c], "idx2": meta["idx2"][c],
             "oh": meta["oh"][c], "Wp": w["Wp"], "bp": w["bp"],
             "W2s": w["W2s"], "g2fm": w["g2fm"], "be2fm": w["be2fm"],
             "cW1": w["cW1"], "cW2": w["cW2"], "cW3": w["cW3"],
             "cb1": w["cb1"], "cb2": w["cb2"], "cb3": w["cb3"]}
        for l in range(3):
            m[f"Wall{l}"] = w[f"Wall{l}"]
        for l in range(2):
            m[f"g{l}"] = w[f"g{l}"]
            m[f"be{l}"] = w[f"be{l}"]
        in_maps.append(m)

    res = bass_utils.run_bass_kernel_spmd(nc, in_maps,
                                          core_ids=list(range(NCORES)))
    out = np.zeros((N, 2), np.float32)
    for c in range(NCORES):
        o = res.results[c]["out"]
        sl = slice(c * NPC, (c + 1) * NPC)
        out[sl] = o[:, pid[sl] - c * NPAD].T
    return out


def run_timed(**inputs):
    """Run once with NTFF tracing; return max per-core exec time in ns."""
    meta = _pack_graph(inputs["edge_index"])
    T, NPAD, NCH = meta["T"], meta["NPAD"], meta["NCH"]
    key = (T, NPAD, NCH)
    if key not in _CACHE:
        _CACHE[key] = _build_program(T, NPAD, NCH)
    nc = _CACHE[key]
    w = _prep_weights(inputs)
    x = np.asarray(inputs["x"], dtype=np.float32)
    pid = meta["pid"]
    NPC = N // NCORES
    in_maps = []
    for c in range(NCORES):
        x_fm = np.zeros((FP, NPAD), np.float32)
        lp = pid[c * NPC:(c + 1) * NPC] - c * NPAD
        x_fm[:F, lp] = x[c * NPC:(c + 1) * NPC].T
        m = {"x_fm": x_fm, "idx1": meta["idx1"][c], "idx2": meta["idx2"][c],
             "oh": meta["oh"][c], "Wp": w["Wp"], "bp": w["bp"],
             "W2s": w["W2s"], "g2fm": w["g2fm"], "be2fm": w["be2fm"],
             "cW1": w["cW1"], "cW2": w["cW2"], "cW3": w["cW3"],
             "cb1": w["cb1"], "cb2": w["cb2"], "cb3": w["cb3"]}
        for l in range(3):
            m[f"Wall{l}"] = w[f"Wall{l}"]
        for l in range(2):
            m[f"g{l}"] = w[f"g{l}"]
            m[f"be{l}"] = w[f"be{l}"]
        in_maps.append(m)
    res = bass_utils.run_bass_kernel_spmd(
        nc, in_maps, core_ids=list(range(NCORES)), trace=True)
    return res.exec_time_ns



# revision 2
# speedup vs baseline: 1.1154x; 1.1154x over previous
"""CHRONOSNet (3-layer GAT + classifier) on 8 Trainium2 NeuronCores.

Sharding: nodes partitioned by destination id across 8 cores (graph
parallel). Per layer each core computes its shard of the node table
(attention-logit columns + features), AllGathers the full table (halo
exchange), dma_gathers per-edge rows, and does segment softmax +
aggregation via 128-edge-contraction matmuls against host-built 0/1
segment matrices. BatchNorm stats are AllReduced. Weights replicated.
Layer 2 (concat=False) aggregates input features per head and applies
W2 after aggregation, avoiding the [N, 2048] tensor.
"""

import sys

sys.path.insert(0, "/opt/trn_rl_repo")

import numpy as np

import concourse.bacc as bacc
import concourse.mybir as mybir
import concourse.tile as tile
import bass_rust
from concourse import bass_utils
from concourse.masks import make_identity

NCORES = 8
N, F, HID, NH, C = 20000, 236, 256, 8, 32
FP = 256
ROWW = 320        # table row: [al_src 8 | al_dst 8 | pad 48 | x 256]
ALOFF = 64
TC = 8            # tiles per gather chunk (chunk = one 128-row psum group)
BN_EPS = 1e-5
DEN_EPS = 1e-16

f32 = mybir.dt.float32
f32r = mybir.dt.float32r
i16 = mybir.dt.int16
AF = mybir.ActivationFunctionType
ALU = mybir.AluOpType


def _split_excess_waits(nc, limit=1):
    """This walrus build rejects >limit sem-waits per instruction; move the
    excess onto same-engine NoOps inserted immediately before the carrier."""
    n_new = 0
    for bb in nc.main_func.blocks:
        insts = bb.instructions
        i = 0
        while i < len(insts):
            ins = insts[i]
            si = ins.sync_info
            if si is not None and si.on_wait and len(si.on_wait) > limit:
                waits = list(si.on_wait)
                extra, keep = waits[:-limit], waits[-limit:]
                new_insts = []
                for j in range(0, len(extra), limit):
                    nsi = bass_rust.SyncInfo(on_wait=extra[j:j + limit],
                                             on_update=[])
                    new_insts.append(mybir.InstNoOp(
                        name=f"I-ws-{n_new}-{i}-{id(ins) % 9973}",
                        engine=ins.engine, ins=[], outs=[], sync_info=nsi))
                    n_new += 1
                ins.sync_info = bass_rust.SyncInfo(
                    on_wait=keep, on_update=list(si.on_update))
                for k, nd in enumerate(new_insts):
                    insts.insert(i + k, nd)
                i += len(new_insts)
            i += 1
    return n_new


# ------------------------------------------------------------- host packing
def _pack_graph(edge_index):
    e0 = np.asarray(edge_index[0], dtype=np.int64)
    e1 = np.asarray(edge_index[1], dtype=np.int64)
    loops = np.arange(N, dtype=np.int64)
    src = np.concatenate([e0, loops])
    dst = np.concatenate([e1, loops])
    order = np.argsort(dst, kind="stable")
    src_s = src[order]
    deg = np.bincount(dst, minlength=N)
    assert deg.max() <= 128, "node degree exceeds one tile"
    starts = np.zeros(N + 1, np.int64)
    np.cumsum(deg, out=starts[1:])

    NPC = N // NCORES
    assign = []
    tmax = 0
    for c in range(NCORES):
        tiles, cur, cur_e = [], [], 0
        for d in range(c * NPC, (c + 1) * NPC):
            g = int(deg[d])
            if cur and (cur_e + g > 128 or len(cur) == 16):
                tiles.append(cur)
                cur, cur_e = [], 0
            cur.append(d)
            cur_e += g
        if cur:
            tiles.append(cur)
        assign.append(tiles)
        tmax = max(tmax, len(tiles))
    # T: multiple of 2*TC so chunks and layer-2 superblocks divide evenly
    T = ((tmax + 2 * TC - 1) // (2 * TC)) * (2 * TC)
    NPAD = T * 16
    NCH = T // TC
    assert NCORES * NPAD < 32768, "padded node ids must fit int16"

    pid = np.full(N, -1, np.int64)
    for c in range(NCORES):
        for t, nodes in enumerate(assign[c]):
            for s, d in enumerate(nodes):
                pid[d] = c * NPAD + t * 16 + s
    assert (pid >= 0).all()

    idx1 = np.zeros((NCORES, T, 128), np.int16)
    idx2 = np.zeros((NCORES, T, 128), np.int16)
    oh = np.zeros((NCORES, 128, T, 16), np.float32)
    for c in range(NCORES):
        for t, nodes in enumerate(assign[c]):
            p = 0
            for s, d in enumerate(nodes):
                for e in range(starts[d], starts[d + 1]):
                    idx1[c, t, p] = pid[src_s[e]]
                    idx2[c, t, p] = pid[d]
                    oh[c, p, t, s] = 1.0
                    p += 1
            assert p <= 128

    def wrap(idx):
        w = np.zeros((NCORES, 128, NCH, TC * 8), np.int16)
        for c in range(NCORES):
            flat = idx[c].reshape(NCH, TC * 128)
            for ch in range(NCH):
                w[c, :, ch, :] = np.tile(flat[ch].reshape(-1, 16).T, (8, 1))
        return w

    return dict(T=T, NPAD=NPAD, NCH=NCH, pid=pid,
                idx1=wrap(idx1), idx2=wrap(idx2), oh=oh)


def _prep_weights(inp):
    w = {}
    Wp = np.zeros((FP, HID), np.float32)
    Wp[:F] = np.asarray(inp["Wp"])
    w["Wp"] = Wp
    w["bp"] = np.asarray(inp["bp"]).reshape(1, HID)

    for l, CC in ((0, C), (1, C), (2, HID)):
        W = np.asarray(inp[f"W{l}"])
        a_s = np.asarray(inp[f"as{l}"])
        a_d = np.asarray(inp[f"ad{l}"])
        As = np.einsum("khc,hc->kh", W.reshape(HID, NH, CC), a_s)
        Ad = np.einsum("khc,hc->kh", W.reshape(HID, NH, CC), a_d)
        wall = np.zeros((HID, ROWW), np.float32)
        wall[:, 0:8] = As
        wall[:, 8:16] = Ad
        wall[:, ALOFF:] = W if l < 2 else np.eye(HID, dtype=np.float32)
        w[f"Wall{l}"] = wall
        w[f"g{l}"] = np.asarray(inp[f"g{l}"]).reshape(1, HID)
        w[f"be{l}"] = np.asarray(inp[f"be{l}"]).reshape(1, HID)

    W2 = np.asarray(inp["W2"]).reshape(HID, NH, HID)
    W2s = np.zeros((128, 16, 2, 128), np.float32)
    for kt in range(16):
        h, kc = kt // 2, kt % 2
        for ch in range(2):
            W2s[:, kt, ch, :] = W2[kc * 128:(kc + 1) * 128, h,
                                   ch * 128:(ch + 1) * 128] / NH
    w["W2s"] = W2s
    # feature-major per-partition forms for layer-2 BN
    g2 = np.asarray(inp["g2"]).reshape(2, 128).T.copy()
    be2 = np.asarray(inp["be2"]).reshape(2, 128).T.copy()
    w["g2fm"], w["be2fm"] = g2, be2

    cW1 = np.asarray(inp["cW1"])
    w["cW1"] = cW1.reshape(4, 128, 2, 128).transpose(1, 0, 2, 3).copy()
    cW2 = np.asarray(inp["cW2"])
    w["cW2"] = cW2.reshape(2, 128, 128).transpose(1, 0, 2).copy()
    w["cW3"] = np.asarray(inp["cW3"])
    w["cb1"] = np.asarray(inp["cb1"]).reshape(2, 128).T.copy()   # [128, 2]
    w["cb2"] = np.asarray(inp["cb2"]).reshape(128, 1).copy()     # [128, 1]
    w["cb3"] = np.asarray(inp["cb3"]).reshape(2, 1).copy()       # [2, 1]
    return w


# ------------------------------------------------------------ device program
def _build_program(T, NPAD, NCH, debug=False):
    NG = NPAD // 128     # 128-row node groups (node-major)
    NB = NPAD // 512     # 512-col node blocks (feature-major)
    NSB = T // (2 * TC)  # layer-2 projection superblocks (16 tiles each)
    nc = bacc.Bacc("TRN2")

    d_x = nc.dram_tensor("x_fm", [FP, NPAD], f32r, kind="ExternalInput")
    d_idx1 = nc.dram_tensor("idx1", [128, NCH, TC * 8], i16, kind="ExternalInput")
    d_idx2 = nc.dram_tensor("idx2", [128, NCH, TC * 8], i16, kind="ExternalInput")
    d_oh = nc.dram_tensor("oh", [128, T, 16], f32r, kind="ExternalInput")
    d_Wp = nc.dram_tensor("Wp", [FP, HID], f32r, kind="ExternalInput")
    d_bp = nc.dram_tensor("bp", [1, HID], f32, kind="ExternalInput")
    d_wall = [nc.dram_tensor(f"Wall{l}", [HID, ROWW], f32r, kind="ExternalInput")
              for l in range(3)]
    d_g = [nc.dram_tensor(f"g{l}", [1, HID], f32, kind="ExternalInput")
           for l in range(2)]
    d_be = [nc.dram_tensor(f"be{l}", [1, HID], f32, kind="ExternalInput")
            for l in range(2)]
    d_g2 = nc.dram_tensor("g2fm", [128, 2], f32, kind="ExternalInput")
    d_be2 = nc.dram_tensor("be2fm", [128, 2], f32, kind="ExternalInput")
    d_W2s = nc.dram_tensor("W2s", [128, 16, 2, 128], f32r, kind="ExternalInput")
    d_cW1 = nc.dram_tensor("cW1", [128, 4, 2, 128], f32r, kind="ExternalInput")
    d_cW2 = nc.dram_tensor("cW2", [128, 2, 128], f32r, kind="ExternalInput")
    d_cW3 = nc.dram_tensor("cW3", [128, 2], f32r, kind="ExternalInput")
    d_cb1 = nc.dram_tensor("cb1", [128, 2], f32, kind="ExternalInput")
    d_cb2 = nc.dram_tensor("cb2", [128, 1], f32, kind="ExternalInput")
    d_cb3 = nc.dram_tensor("cb3", [2, 1], f32, kind="ExternalInput")
    d_out = nc.dram_tensor("out", [2, NPAD], f32, kind="ExternalOutput")
    if debug:
        d_dbg_h0 = nc.dram_tensor("dbg_h0", [128, NPAD // 128, HID], f32,
                                  kind="ExternalOutput")
        d_dbg_u0 = nc.dram_tensor("dbg_u0", [128, NPAD // 128, 264], f32,
                                  kind="ExternalOutput")
        d_dbg_h1 = nc.dram_tensor("dbg_h1", [128, NPAD // 128, HID], f32,
                                  kind="ExternalOutput")
        d_dbg_h2 = nc.dram_tensor("dbg_h2", [128, NPAD // 128, HID], f32,
                                  kind="ExternalOutput")
        d_dbg_u2 = nc.dram_tensor("dbg_u2", [128, 2, NPAD], f32,
                                  kind="ExternalOutput")
        d_dbg_hf = nc.dram_tensor("dbg_hf", [128, 2, NPAD], f32,
                                  kind="ExternalOutput")

    RG = [list(range(NCORES))]

    with tile.TileContext(nc) as tc, \
         tc.tile_pool(name="const", bufs=1) as const, \
         tc.tile_pool(name="sb", bufs=2) as sbuf, \
         tc.tile_pool(name="sb1", bufs=1) as sb1, \
         tc.tile_pool(name="big", bufs=1) as big, \
         tc.tile_pool(name="psA", bufs=2, space="PSUM") as psA, \
         tc.tile_pool(name="psB", bufs=2, space="PSUM") as psB, \
         tc.tile_pool(name="dram", bufs=1, space="DRAM") as dram:

        # ---------------- constants
        ident_t = const.tile([128, 128], f32, tag="ident")
        make_identity(nc, ident_t[:])
        ident = ident_t[:]
        ones_t = const.tile([128, 1], f32, tag="ones")
        nc.gpsimd.memset(ones_t[:], 1.0)
        ones_col = ones_t[:].bitcast(f32r)
        idx1_sb = const.tile([128, NCH, TC * 8], i16, tag="idx1")
        nc.sync.dma_start(idx1_sb[:], d_idx1[:])
        idx2_sb = const.tile([128, NCH, TC * 8], i16, tag="idx2")
        nc.sync.dma_start(idx2_sb[:], d_idx2[:])
        Wp_sb = const.tile([128, 2, HID], f32r, tag="Wp")
        nc.sync.dma_start(Wp_sb[:],
                          d_Wp.ap().rearrange("(kc k) w -> k kc w", k=128))
        W2s_sb = const.tile([128, 16, 2, 128], f32r, tag="W2s")
        nc.sync.dma_start(W2s_sb[:], d_W2s[:])
        cW1_sb = const.tile([128, 4, 2, 128], f32r, tag="cW1")
        nc.sync.dma_start(cW1_sb[:], d_cW1[:])
        cW2_sb = const.tile([128, 2, 128], f32r, tag="cW2")
        nc.sync.dma_start(cW2_sb[:], d_cW2[:])
        cW3_sb = const.tile([128, 2], f32r, tag="cW3")
        nc.sync.dma_start(cW3_sb[:], d_cW3[:])
        gam_sb = const.tile([1, 2, HID], f32, tag="gam")
        bet_sb = const.tile([1, 2, HID], f32, tag="bet")
        for l in range(2):
            nc.sync.dma_start(gam_sb[:, l, :], d_g[l][:])
            nc.sync.dma_start(bet_sb[:, l, :], d_be[l][:])
        g2_sb = const.tile([128, 2], f32, tag="g2f")
        nc.sync.dma_start(g2_sb[:], d_g2[:])
        be2_sb = const.tile([128, 2], f32, tag="be2f")
        nc.sync.dma_start(be2_sb[:], d_be2[:])
        cb1_sb = const.tile([128, 2], f32, tag="cb1")
        nc.sync.dma_start(cb1_sb[:], d_cb1[:])
        cb2_sb = const.tile([128, 1], f32, tag="cb2")
        nc.sync.dma_start(cb2_sb[:], d_cb2[:])
        cb3_sb = const.tile([2, 1], f32, tag="cb3")
        nc.sync.dma_start(cb3_sb[:], d_cb3[:])
        bp1 = const.tile([1, HID], f32, tag="bp1")
        nc.sync.dma_start(bp1[:], d_bp[:])
        bp_rep = const.tile([128, HID], f32, tag="bpr")
        nc.gpsimd.partition_broadcast(bp_rep[:], bp1[:])

        # ---------------- persistent state
        h_nm = big.tile([128, NG, HID], f32r, tag="h_nm")   # current h
        ht_dram = dram.tile([HID, NPAD], f32r, tag="ht")    # h_temporal (fm)

        # ---------------- input projection: h = x @ Wp + bp (node-major)
        for g in range(NG):
            xst = sbuf.tile([128, 2, 128], f32r, tag="xst")
            nc.sync.dma_start(xst[:], d_x.ap().rearrange(
                "(kc k) n -> k kc n", k=128)[:, :, g * 128:(g + 1) * 128])
            pm = psA.tile([128, 512], f32, tag="pA")
            for kc in range(2):
                nc.tensor.matmul(pm[:, 0:HID], xst[:, kc, :], Wp_sb[:, kc, :],
                                 start=(kc == 0), stop=(kc == 1))
            nc.vector.tensor_add(h_nm[:, g, :], pm[:, 0:HID], bp_rep[:])
            # h_temporal, feature-major, to DRAM
            for kc in range(2):
                pt = psB.tile([128, 128], f32, tag="pB")
                nc.tensor.transpose(pt[:], h_nm[:, g, kc * 128:(kc + 1) * 128].bitcast(f32),
                                    ident)
                hst = sbuf.tile([128, 128], f32r, tag="hst")
                nc.vector.tensor_copy(hst[:], pt[:])
                nc.sync.dma_start(
                    ht_dram[kc * 128:(kc + 1) * 128, g * 128:(g + 1) * 128],
                    hst[:])

        if debug:
            nc.sync.dma_start(d_dbg_h0[:], h_nm[:].bitcast(f32))

        def stat_allreduce(stats, tag):
            if not isinstance(stats, list):
                stats = [stats]
            n = len(stats)
            w = stats[0].shape[-1]
            p = stats[0].shape[0]
            sin = dram.tile([n * p, w], f32, tag=f"ari{tag}")
            sout = dram.tile([n * p, w], f32, tag=f"aro{tag}")
            for k, s in enumerate(stats):
                nc.sync.dma_start(sin[k * p:(k + 1) * p, :], s[:])
            nc.gpsimd.collective_compute(
                "AllReduce", ALU.add, ins=[sin.opt()], outs=[sout.opt()],
                replica_groups=RG)
            for k, s in enumerate(stats):
                nc.sync.dma_start(s[:], sout[k * p:(k + 1) * p, :])

        # ================= GAT layers =================
        for l in range(3):
            # ---- shard of the node table: xrow = h.T-block @ Wall_l
            wall_sb = sbuf.tile([128, 2, ROWW], f32r, tag="wall")
            nc.sync.dma_start(
                wall_sb[:],
                d_wall[l].ap().rearrange("(kc k) w -> k kc w", k=128))
            xloc = dram.tile([NPAD, ROWW], f32r, tag=f"xloc{l}")
            for g in range(NG):
                hTst = sbuf.tile([128, 2, 128], f32r, tag="hTst")
                for kc in range(2):
                    pt = psB.tile([128, 128], f32, tag="pB")
                    nc.tensor.transpose(
                        pt[:], h_nm[:, g, kc * 128:(kc + 1) * 128].bitcast(f32),
                        ident)
                    nc.vector.tensor_copy(hTst[:, kc, :], pt[:])
                pm = psA.tile([128, 512], f32, tag="pA")
                for kc in range(2):
                    nc.tensor.matmul(pm[:, 0:ROWW], hTst[:, kc, :],
                                     wall_sb[:, kc, :],
                                     start=(kc == 0), stop=(kc == 1))
                xr = sbuf.tile([128, ROWW], f32r, tag="xrow")
                nc.vector.tensor_copy(xr[:], pm[:, 0:ROWW])
                nc.sync.dma_start(xloc[g * 128:(g + 1) * 128, :], xr[:])
            xtab = dram.tile([NCORES * NPAD, ROWW], f32r, tag=f"xtab{l}",
                             addr_space="Shared")
            nc.gpsimd.collective_compute(
                "AllGather", ALU.bypass, ins=[xloc.opt()], outs=[xtab.opt()],
                replica_groups=RG)

            # ---- gather chunks + segment softmax + aggregation
            if l < 2:
                u_sb = big.tile([128, NG, 264], f32, tag="u_sb")
            else:
                u2_fm = big.tile([128, 2, NPAD], f32, tag="u_sb")
            g2l = None
            for ch in range(NCH):
                ohc = sbuf.tile([128, TC, 16], f32r, tag="ohc")
                nc.sync.dma_start(ohc[:], d_oh[:, ch * TC:(ch + 1) * TC, :])
                gt = sbuf.tile([128, TC, ROWW], f32r, tag="gmain")
                nc.gpsimd.dma_gather(gt[:], xtab[:], idx1_sb[:, ch, :],
                                     TC * 128, TC * 128, ROWW)
                g2t = sbuf.tile([128, TC, 64], f32, tag="gal")
                nc.gpsimd.dma_gather(g2t[:], xtab.opt()[:, 0:64].bitcast(f32),
                                     idx2_sb[:, ch, :], TC * 128, TC * 128, 64,
                                     elem_step=ROWW)
                lg = sbuf.tile([128, TC, 8], f32, tag="lg")
                nc.vector.tensor_add(lg[:], gt[:, :, 0:8].bitcast(f32),
                                     g2t[:, :, 8:16])
                nc.vector.scalar_tensor_tensor(lg[:], lg[:], 0.2, lg[:],
                                               ALU.mult, ALU.max)
                ex = sbuf.tile([128, TC, 8], f32, tag="ex")
                nc.scalar.activation(ex[:], lg[:], AF.Exp)
                if l < 2:
                    y = sbuf.tile([128, TC, 264], f32r, tag="y")
                    nc.vector.tensor_mul(
                        y[:, :, 0:256].rearrange("p t (h c) -> p t h c", h=8),
                        gt[:, :, ALOFF:].bitcast(f32)
                          .rearrange("p t (h c) -> p t h c", h=8),
                        ex[:].unsqueeze(3).broadcast_to([128, TC, 8, 32]))
                    nc.vector.tensor_copy(y[:, :, 256:264], ex[:])
                    ohg = sb1.tile([128, TC, 128], f32r, tag="ohg")
                    nc.vector.memset(ohg[:].bitcast(f32), 0.0)
                    for j in range(TC):
                        nc.vector.tensor_copy(
                            ohg[:, j, j * 16:(j + 1) * 16], ohc[:, j, :])
                    pu = psA.tile([128, 512], f32, tag="pA")
                    for j in range(TC):
                        nc.tensor.matmul(pu[:, 0:264], ohg[:, j, :],
                                         y[:, j, :], start=(j == 0),
                                         stop=(j == TC - 1))
                    nc.vector.tensor_copy(u_sb[:, ch, :].bitcast(f32r), pu[:, 0:264])
                else:
                    ohex = sbuf.tile([128, TC, 16, 8], f32r, tag="y")
                    nc.vector.tensor_mul(
                        ohex[:],
                        ohc[:].unsqueeze(3).broadcast_to([128, TC, 16, 8]),
                        ex[:].unsqueeze(2).broadcast_to([128, TC, 16, 8]))
                    if ch % 2 == 0:
                        g2l = big.tile([128, 2 * TC, 2, 16, 8], f32r,
                                       tag="g2l")
                    for j in range(TC):
                        tt = (ch % 2) * TC + j
                        pg = psA.tile([128, 512], f32, tag="pA")
                        lhs = ohex[:, j, :, :].rearrange("p s h -> p (s h)")
                        nc.tensor.matmul(pg[:, 0:256], lhs, gt[:, j, ALOFF:],
                                         start=True, stop=True)
                        nc.tensor.matmul(pg[:, 256:257],
                                         lhs.bitcast(f32), ones_t[:],
                                         start=True, stop=True)
                        rden = sbuf.tile([128, 1], f32, tag="rden")
                        nc.vector.tensor_scalar_add(rden[:], pg[:, 256:257],
                                                    DEN_EPS)
                        nc.vector.reciprocal(rden[:], rden[:])
                        gn = sbuf.tile([128, 256], f32r, tag="gn")
                        nc.vector.tensor_scalar(gn[:], pg[:, 0:256], rden[:],
                                                None, ALU.mult)
                        for kc in range(2):
                            pt = psB.tile([128, 128], f32, tag="pB")
                            nc.tensor.transpose(
                                pt[:], gn[:, kc * 128:(kc + 1) * 128]
                                    .bitcast(f32), ident)
                            nc.vector.tensor_copy(
                                g2l[:, tt, kc, :, :]
                                    .rearrange("p s h -> p (s h)"), pt[:])
                    if ch % 2 == 1:
                        sb_i = ch // 2
                        for chh in range(2):
                            pm = psA.tile([128, 512], f32, tag="pA")
                            for kt in range(16):
                                rhs = g2l[:, :, kt % 2, :, kt // 2]
                                nc.tensor.matmul(
                                    pm[:, 0:256], W2s_sb[:, kt, chh, :], rhs,
                                    start=(kt == 0), stop=(kt == 15))
                            nc.vector.tensor_copy(
                                u2_fm[:, chh, sb_i * 256:(sb_i + 1) * 256]
                                    .bitcast(f32r),
                                pm[:, 0:256])

            if debug and l == 0:
                nc.sync.dma_start(d_dbg_u0[:], u_sb[:].bitcast(f32))
            if debug and l == 2:
                nc.sync.dma_start(d_dbg_u2[:], u2_fm[:])
            # ---- postprocess
            if l < 2:
                rden = sbuf.tile([128, NG, 8], f32, tag="rdnA")
                nc.vector.tensor_scalar_add(rden[:], u_sb[:, :, 256:264],
                                            DEN_EPS)
                nc.vector.reciprocal(rden[:], rden[:])
                ubv = u_sb[:, :, 0:256].bitcast(f32r)
                nc.vector.tensor_mul(
                    ubv.rearrange("p g (h c) -> p g h c", h=8),
                    ubv.rearrange("p g (h c) -> p g h c", h=8),
                    rden[:].unsqueeze(3).broadcast_to([128, NG, 8, 32]))
                # stats: sum and sumsq over all node slots (dummies are 0)
                pst = psA.tile([128, 512], f32, tag="pA")
                pst2 = psA.tile([128, 512], f32, tag="pA")
                for g in range(NG):
                    nc.tensor.matmul(pst[0:1, 0:HID], ones_col,
                                     ubv[:, g, :], start=(g == 0),
                                     stop=(g == NG - 1))
                for b in range(NG // 4):
                    sq = sb1.tile([128, 4, 256], f32r, tag="scr1")
                    nc.vector.tensor_mul(sq[:], ubv[:, b * 4:(b + 1) * 4, :],
                                         ubv[:, b * 4:(b + 1) * 4, :])
                    for gg in range(4):
                        g = b * 4 + gg
                        nc.tensor.matmul(pst2[0:1, 0:HID], ones_col,
                                         sq[:, gg, :], start=(g == 0),
                                         stop=(g == NG - 1))
                stat_s = sbuf.tile([1, HID], f32, tag="stat_s")
                stat_q = sbuf.tile([1, HID], f32, tag="stat_q")
                nc.vector.tensor_copy(stat_s[:], pst[0:1, 0:HID])
                nc.vector.tensor_copy(stat_q[:], pst2[0:1, 0:HID])
                stat_allreduce([stat_s, stat_q], f"l{l}")
                mu = sbuf.tile([1, HID], f32, tag="mu")
                nc.vector.tensor_scalar_mul(mu[:], stat_s[:], 1.0 / N)
                var = sbuf.tile([1, HID], f32, tag="var")
                nc.vector.tensor_scalar_mul(var[:], stat_q[:], 1.0 / N)
                musq = sbuf.tile([1, HID], f32, tag="musq")
                nc.vector.tensor_mul(musq[:], mu[:], mu[:])
                nc.vector.tensor_tensor(var[:], var[:], musq[:],
                                        op=ALU.subtract)
                rstd = sbuf.tile([1, HID], f32, tag="rstd")
                nc.vector.tensor_scalar_add(var[:], var[:], BN_EPS)
                nc.scalar.activation(rstd[:], var[:], AF.Sqrt)
                nc.vector.reciprocal(rstd[:], rstd[:])
                A1 = sbuf.tile([1, HID], f32, tag="A1")
                nc.vector.tensor_mul(A1[:], rstd[:], gam_sb[:, l, :])
                B1 = sbuf.tile([1, HID], f32, tag="B1")
                nc.vector.tensor_mul(B1[:], mu[:], A1[:])
                nc.vector.tensor_tensor(B1[:], bet_sb[:, l, :], B1[:],
                                        op=ALU.subtract)
                Ar = sb1.tile([128, HID], f32, tag="Ar")
                nc.gpsimd.partition_broadcast(Ar[:], A1[:])
                Br = sb1.tile([128, HID], f32, tag="Br")
                nc.gpsimd.partition_broadcast(Br[:], B1[:])
                for b in range(NG // 4):
                    sl = slice(b * 4, (b + 1) * 4)
                    bn = sb1.tile([128, 4, 256], f32, tag="scr1")
                    nc.vector.tensor_mul(
                        bn[:], ubv[:, sl, :],
                        Ar[:].unsqueeze(1).broadcast_to([128, 4, HID]))
                    nc.vector.tensor_add(
                        bn[:], bn[:],
                        Br[:].unsqueeze(1).broadcast_to([128, 4, HID]))
                    # elu(x) = relu(x) + exp(min(x,0)) - 1
                    r_ = sb1.tile([128, 4, 256], f32, tag="scr2")
                    nc.scalar.activation(r_[:], bn[:], AF.Relu)
                    nc.vector.tensor_scalar_min(bn[:], bn[:], 0.0)
                    nc.scalar.activation(bn[:], bn[:], AF.Exp)
                    nc.vector.tensor_add(bn[:], bn[:], r_[:])
                    # h_new = (bn - 1) + h_prev, in place on h_nm
                    nc.vector.scalar_tensor_tensor(
                        h_nm[:, sl, :], bn[:], -1.0, h_nm[:, sl, :],
                        ALU.add, ALU.add)
                if debug:
                    nc.sync.dma_start((d_dbg_h1 if l == 0 else d_dbg_h2)[:],
                                      h_nm[:].bitcast(f32))
            else:
                # ---- BN2 (feature-major; dummy cols are exactly 0)
                st2 = sbuf.tile([128, 4], f32, tag="st2")
                nc.vector.reduce_sum(st2[:, 0:2].unsqueeze(2), u2_fm[:],
                                     axis=mybir.AxisListType.X)
                sq2 = big.tile([128, 2, NPAD], f32, tag="g2l")
                nc.vector.tensor_mul(sq2[:], u2_fm[:], u2_fm[:])
                nc.vector.reduce_sum(st2[:, 2:4].unsqueeze(2), sq2[:],
                                     axis=mybir.AxisListType.X)
                stat_allreduce(st2, "l2")
                mu2 = sbuf.tile([128, 2], f32, tag="mu2")
                nc.vector.tensor_scalar_mul(mu2[:], st2[:, 0:2], 1.0 / N)
                var2 = sbuf.tile([128, 2], f32, tag="var2")
                nc.vector.tensor_scalar_mul(var2[:], st2[:, 2:4], 1.0 / N)
                m2sq = sbuf.tile([128, 2], f32, tag="m2sq")
                nc.vector.tensor_mul(m2sq[:], mu2[:], mu2[:])
                nc.vector.tensor_tensor(var2[:], var2[:], m2sq[:],
                                        op=ALU.subtract)
                rstd2 = sbuf.tile([128, 2], f32, tag="rstd2")
                nc.vector.tensor_scalar_add(var2[:], var2[:], BN_EPS)
                nc.scalar.activation(rstd2[:], var2[:], AF.Sqrt)
                nc.vector.reciprocal(rstd2[:], rstd2[:])
                A2 = sbuf.tile([128, 2], f32, tag="A2")
                nc.vector.tensor_mul(A2[:], rstd2[:], g2_sb[:])
                B2 = sbuf.tile([128, 2], f32, tag="B2")
                nc.vector.tensor_mul(B2[:], mu2[:], A2[:])
                nc.vector.tensor_tensor(B2[:], be2_sb[:], B2[:],
                                        op=ALU.subtract)
                hfin = u2_fm[:].bitcast(f32r)
                for chh in range(2):
                    nc.vector.tensor_scalar(
                        hfin[:, chh, :], u2_fm[:, chh, :],
                        A2[:, chh:chh + 1], B2[:, chh:chh + 1],
                        ALU.mult, ALU.add)

        if debug:
            nc.sync.dma_start(d_dbg_hf[:], u2_fm[:])
        # ================= classifier (feature-major) =================
        z1 = big.tile([128, 2, NPAD], f32r, tag="g2l")
        for b in range(NB):
            htst = sb1.tile([128, 2, 512], f32r, tag="htst")
            nc.sync.dma_start(htst[:], ht_dram.opt().rearrange(
                "(kc k) n -> k kc n", k=128)[:, :, b * 512:(b + 1) * 512])
            for mh in range(2):
                pm = psA.tile([128, 512], f32, tag="pA")
                for kc in range(4):
                    rhs = (hfin[:, kc, b * 512:(b + 1) * 512] if kc < 2
                           else htst[:, kc - 2, :])
                    nc.tensor.matmul(pm[:], cW1_sb[:, kc, mh, :], rhs,
                                     start=(kc == 0), stop=(kc == 3))
                nc.vector.tensor_scalar(
                    z1[:, mh, b * 512:(b + 1) * 512], pm[:],
                    cb1_sb[:, mh:mh + 1], 0.0, ALU.add, ALU.max)
        z2 = big.tile([128, NPAD], f32r, tag="h_nm")
        for b in range(NB):
            pm = psA.tile([128, 512], f32, tag="pA")
            for kc in range(2):
                nc.tensor.matmul(pm[:], cW2_sb[:, kc, :],
                                 z1[:, kc, b * 512:(b + 1) * 512],
                                 start=(kc == 0), stop=(kc == 1))
            nc.vector.tensor_scalar(z2[:, b * 512:(b + 1) * 512], pm[:],
                                    cb2_sb[:], 0.0, ALU.add, ALU.max)
        for b in range(NB):
            pm = psA.tile([128, 512], f32, tag="pA")
            nc.tensor.matmul(pm[0:2, :], cW3_sb[:],
                             z2[:, b * 512:(b + 1) * 512],
                             start=True, stop=True)
            z3 = sbuf.tile([2, 512], f32, tag="z3")
            nc.vector.tensor_scalar_add(z3[:], pm[0:2, :], cb3_sb[:])
            nc.sync.dma_start(d_out.ap()[:, b * 512:(b + 1) * 512], z3[:])

    nc.compile()
    _split_excess_waits(nc)
    return nc


_CACHE = {}


def kernel(**inputs):
    meta = _pack_graph(inputs["edge_index"])
    T, NPAD, NCH = meta["T"], meta["NPAD"], meta["NCH"]
    key = (T, NPAD, NCH)
    if key not in _CACHE:
        _CACHE[key] = _build_program(T, NPAD, NCH)
    nc = _CACHE[key]

    w = _prep_weights(inputs)
    x = np.asarray(inputs["x"], dtype=np.float32)
    pid = meta["pid"]
    NPC = N // NCORES

    in_maps = []
    for c in range(NCORES):
        x_fm = np.zeros((FP, NPAD), np.float32)
        lp = pid[c * NPC:(c + 1) * NPC] - c * NPAD
        x_fm[:F, lp] = x[c * NPC:(c + 1) * NPC].T
        m = {"x_fm": x_fm, "idx1": meta["idx1"][c], "idx2": meta["idx2"][c],
             "oh": meta["oh"][c], "Wp": w["Wp"], "bp": w["bp"],
             "W2s": w["W2s"], "g2fm": w["g2fm"], "be2fm": w["be2fm"],
             "cW1": w["cW1"], "cW2": w["cW2"], "cW3": w["cW3"],
             "cb1": w["cb1"], "cb2": w["cb2"], "cb3": w["cb3"]}
        for l in range(3):
            m[f"Wall{l}"] = w[f"Wall{l}"]
        for l in range(2):
            m[f"g{l}"] = w[f"g{l}"]
            m[f"be{l}"] = w[f"be{l}"]
        in_maps.append(m)

    res = bass_utils.run_bass_kernel_spmd(nc, in_maps,
                                          core_ids=list(range(NCORES)))
    out = np.zeros((N, 2), np.float32)
    for c in range(NCORES):
        o = res.results[c]["out"]
        sl = slice(c * NPC, (c + 1) * NPC)
        out[sl] = o[:, pid[sl] - c * NPAD].T
    return out


def run_timed(**inputs):
    """Run once with NTFF tracing; return max per-core exec time in ns."""
    meta = _pack_graph(inputs["edge_index"])
    T, NPAD, NCH = meta["T"], meta["NPAD"], meta["NCH"]
    key = (T, NPAD, NCH)
    if key not in _CACHE:
        _CACHE[key] = _build_program(T, NPAD, NCH)
    nc = _CACHE[key]
    w = _prep_weights(inputs)
    x = np.asarray(inputs["x"], dtype=np.float32)
    pid = meta["pid"]
    NPC = N // NCORES
    in_maps = []
    for c in range(NCORES):
        x_fm = np.zeros((FP, NPAD), np.float32)
        lp = pid[c * NPC:(c + 1) * NPC] - c * NPAD
        x_fm[:F, lp] = x[c * NPC:(c + 1) * NPC].T
        m = {"x_fm": x_fm, "idx1": meta["idx1"][c], "idx2": meta["idx2"][c],
             "oh": meta["oh"][c], "Wp": w["Wp"], "bp": w["bp"],
             "W2s": w["W2s"], "g2fm": w["g2fm"], "be2fm": w["be2fm"],
             "cW1": w["cW1"], "cW2": w["cW2"], "cW3": w["cW3"],
             "cb1": w["cb1"], "cb2": w["cb2"], "cb3": w["cb3"]}
        for l in range(3):
            m[f"Wall{l}"] = w[f"Wall{l}"]
        for l in range(2):
            m[f"g{l}"] = w[f"g{l}"]
            m[f"be{l}"] = w[f"be{l}"]
        in_maps.append(m)
    res = bass_utils.run_bass_kernel_spmd(
        nc, in_maps, core_ids=list(range(NCORES)), trace=True)
    return res.exec_time_ns

